# revision 70
# baseline (speedup 1.0000x reference)
"""Trainium2 Bass kernel for nn_Attention (B=4, SEQ=2048, DIM=1024, H=16).

Sharding: tensor-parallel over heads — 2 heads per core on 8 cores.
Per core: QKV projection (its heads), attention, row-parallel FC partial.
Gather: host sums the 8 partial FC outputs (+ b_fc).

Device layout notes:
- All projection/score matmuls run in float32r (full-rate fp32 PE mode).
- Scores are computed transposed (S^T: keys on partitions, queries free) so
  softmax(P^T) feeds the AV matmul directly as the moving operand.
- Padding mask is folded into an augmented V operand: column 64 of each
  v-tile holds keep[k] (0/1) and v rows are pre-scaled by keep[k], so
  exp needs no mask bias and the attention row-sum falls out of the same
  matmul (output row 64).
- Normalization (1/rowsum) is applied between AV and FC via a rank-1
  PE-broadcast of the reciprocal row.
"""

import sys

sys.path.insert(0, "/opt/trn_rl_repo")

from contextlib import ExitStack

import numpy as np

import concourse.bass as bass
import concourse.tile as tile
from concourse import bacc, mybir
from concourse.bass_utils import run_bass_kernel_spmd

F32 = mybir.dt.float32
F32R = mybir.dt.float32r
BF16 = mybir.dt.bfloat16

B, SEQ, DIM, H, DH = 4, 2048, 1024, 16, 64
ROWS = B * SEQ  # 8192
SCALE = DH ** -0.5  # 0.125

_CACHE = {}
LAST_RESULTS = None


def _build():
    nc = bacc.Bacc(
        "TRN2",
        target_bir_lowering=False,
        debug=False,
        enable_asserts=False,
        num_devices=8,
    )
    xT = nc.dram_tensor("xT", [DIM, ROWS], F32R, kind="ExternalInput").ap()
    wqkvT = nc.dram_tensor("wqkvT", [DIM, 384], F32R, kind="ExternalInput").ap()
    wfcT = nc.dram_tensor("wfcT", [128, DIM], F32R, kind="ExternalInput").ap()
    keep = nc.dram_tensor("keep", [B, 128, 16], F32, kind="ExternalInput").ap()
    id128 = nc.dram_tensor("id128", [128, 128], F32, kind="ExternalInput").ap()
    eA = nc.dram_tensor("eA", [1, 128], F32R, kind="ExternalInput").ap()
    eB = nc.dram_tensor("eB", [1, 128], F32R, kind="ExternalInput").ap()
    outp = nc.dram_tensor("outp", [ROWS, DIM], F32, kind="ExternalOutput").ap()

    EXP = mybir.ActivationFunctionType.Exp

    with tile.TileContext(nc) as tc, ExitStack() as ctx:
        p_const = ctx.enter_context(tc.tile_pool(name="const", bufs=1))
        p_xin = ctx.enter_context(tc.tile_pool(name="xin", bufs=10))
        p_qk = ctx.enter_context(tc.tile_pool(name="qk", bufs=1))
        p_vt = ctx.enter_context(tc.tile_pool(name="vt", bufs=1))
        p_va = ctx.enter_context(tc.tile_pool(name="va", bufs=2))
        p_pt = ctx.enter_context(tc.tile_pool(name="pt", bufs=40))
        p_xn = ctx.enter_context(tc.tile_pool(name="xn", bufs=3))
        p_rsb = ctx.enter_context(tc.tile_pool(name="rsb", bufs=2))
        p_r = ctx.enter_context(tc.tile_pool(name="r", bufs=1))
        p_fco = ctx.enter_context(tc.tile_pool(name="fco", bufs=3))
        p_st = ctx.enter_context(tc.tile_pool(name="st", bufs=2, space="PSUM"))
        p_xa = ctx.enter_context(tc.tile_pool(name="xa", bufs=2, space="PSUM"))
        p_mm = ctx.enter_context(tc.tile_pool(name="mm", bufs=2, space="PSUM"))

        wqkv_sb = p_const.tile([128, 8 * 384], F32R, tag="wqkv")
        for c in range(8):
            nc.sync.dma_start(
                wqkv_sb[:, c * 384 : (c + 1) * 384],
                wqkvT[c * 128 : (c + 1) * 128, :],
            )
        wfc_sb = p_const.tile([128, DIM], F32R, tag="wfc")
        nc.sync.dma_start(wfc_sb[:], wfcT[:])
        keep_sb = p_const.tile([128, 64], F32, tag="keep")
        for b in range(B):
            nc.sync.dma_start(keep_sb[:, b * 16 : (b + 1) * 16], keep[b])
        id_sb = p_const.tile([128, 128], F32, tag="id")
        nc.sync.dma_start(id_sb[:], id128[:])
        ea_sb = p_const.tile([1, 128], F32R, tag="ea")
        nc.sync.dma_start(ea_sb[:], eA[:])
        eb_sb = p_const.tile([1, 128], F32R, tag="eb")
        nc.sync.dma_start(eb_sb[:], eB[:])

        for b in range(B):
            # ---- QKV projection: qT/kT/vT [128ch, 2048rows] for this batch
            qT2 = p_qk.tile([128, SEQ], F32R, tag="q")
            kT2 = p_qk.tile([128, SEQ], F32R, tag="k")
            vT2 = p_vt.tile([128, SEQ], F32, tag="v")
            dsts = [qT2, kT2, vT2]
            for n in range(4):
                xins = []
                for c in range(8):
                    xt = p_xin.tile([128, 512], F32R, tag="xin")
                    nc.sync.dma_start(
                        xt[:],
                        xT[
                            c * 128 : (c + 1) * 128,
                            b * SEQ + n * 512 : b * SEQ + (n + 1) * 512,
                        ],
                    )
                    xins.append(xt)
                for m in range(3):
                    ps = p_mm.tile([128, 512], F32, tag="mm")
                    for c in range(8):
                        nc.tensor.matmul(
                            ps[:],
                            wqkv_sb[
                                :, c * 384 + m * 128 : c * 384 + (m + 1) * 128
                            ],
                            xins[c][:],
                            start=(c == 0),
                            stop=(c == 7),
                        )
                    nc.vector.tensor_copy(dsts[m][:, n * 512 : (n + 1) * 512], ps[:])

            # ---- v-transpose + keep-scaled augmented V  [128k, 16*(65+65)] bf16
            va = p_va.tile([128, 16 * 130], BF16, tag="va")
            for kj in range(16):
                tp = p_mm.tile([128, 128], F32, tag="mm")
                nc.tensor.transpose(tp[:], vT2[:, kj * 128 : (kj + 1) * 128], id_sb[:])
                kap = keep_sb[:, b * 16 + kj : b * 16 + kj + 1]
                o = kj * 130
                nc.vector.tensor_scalar_mul(va[:, o : o + 64], tp[:, 0:64], kap)
                nc.vector.tensor_copy(va[:, o + 64 : o + 65], kap)
                nc.vector.tensor_scalar_mul(va[:, o + 65 : o + 129], tp[:, 64:128], kap)
                nc.vector.tensor_copy(va[:, o + 129 : o + 130], kap)

            # ---- attention + FC per 1024-query tile
            for qt in range(2):
                q0 = qt * 1024
                pts = {}
                for a in range(2):
                    for kj in range(16):
                        st = p_st.tile([128, 1024], F32, tag="st")
                        for hh in range(2):
                            nc.tensor.matmul(
                                st[:, hh * 512 : (hh + 1) * 512],
                                kT2[
                                    a * 64 : (a + 1) * 64, kj * 128 : (kj + 1) * 128
                                ],
                                qT2[
                                    a * 64 : (a + 1) * 64,
                                    q0 + hh * 512 : q0 + (hh + 1) * 512,
                                ],
                                start=True,
                                stop=True,
                            )
                        pt = p_pt.tile([128, 1024], BF16, tag="pt")
                        nc.scalar.activation(pt[:], st[:], EXP, scale=SCALE)
                        pts[(a, kj)] = pt

                for qh in range(2):
                    xaugs = []
                    for a in range(2):
                        xa = p_xa.tile([65, 512], F32, tag="xa")
                        for kj in range(16):
                            o = kj * 130 + a * 65
                            nc.tensor.matmul(
                                xa[:],
                                va[:, o : o + 65],
                                pts[(a, kj)][:, qh * 512 : (qh + 1) * 512],
                                start=(kj == 0),
                                stop=(kj == 15),
                            )
                        xaugs.append(xa)
                    # stage PSUM->SBUF (DMA cannot read PSUM)
                    xasA = p_xn.tile([65, 512], F32, tag="xasA")
                    nc.vector.tensor_copy(xasA[:], xaugs[0][:])
                    xasB = p_xn.tile([65, 512], F32, tag="xasB")
                    nc.vector.tensor_copy(xasB[:], xaugs[1][:])
                    # normalization: R[p,q] = 1/rowsum of head(p)
                    rA = p_r.tile([1, 512], F32, tag="ra")
                    nc.sync.dma_start(rA[:], xasA[64:65, :])
                    rB = p_r.tile([1, 512], F32, tag="rb")
                    nc.sync.dma_start(rB[:], xasB[64:65, :])
                    rAi = p_r.tile([1, 512], F32, tag="rai")
                    nc.vector.reciprocal_approx_fast(rAi[:], rA[:])
                    rBi = p_r.tile([1, 512], F32, tag="rbi")
                    nc.vector.reciprocal_approx_fast(rBi[:], rB[:])
                    rAc = p_r.tile([1, 512], F32R, tag="rac")
                    nc.vector.tensor_copy(rAc[:], rAi[:])
                    rBc = p_r.tile([1, 512], F32R, tag="rbc")
                    nc.vector.tensor_copy(rBc[:], rBi[:])
                    Rp = p_mm.tile([128, 512], F32, tag="mm")
                    nc.tensor.matmul(
                        Rp[:], ea_sb[:], rAc[:],
                        start=True, stop=False,
                    )
                    nc.tensor.matmul(
                        Rp[:], eb_sb[:], rBc[:],
                        start=False, stop=True,
                    )
                    Rs = p_rsb.tile([128, 512], F32R, tag="rs")
                    nc.vector.tensor_copy(Rs[:], Rp[:])
                    xn = p_xn.tile([128, 512], F32R, tag="xn")
                    nc.vector.tensor_copy(xn[0:64, :], xasA[0:64, :])
                    nc.sync.dma_start(xn[64:128, :].bitcast(F32), xasB[0:64, :])
                    nc.vector.tensor_mul(xn[:], xn[:], Rs[:])

                    # FC partial: out[q,:] = xn^T @ wfcT
                    for qq in range(4):
                        fo = p_fco.tile([128, DIM], F32, tag="fo")
                        for ot in range(2):
                            fp_ = p_mm.tile([128, 512], F32, tag="mm")
                            nc.tensor.matmul(
                                fp_[:],
                                xn[:, qq * 128 : (qq + 1) * 128],
                                wfc_sb[:, ot * 512 : (ot + 1) * 512],
                                start=True,
                                stop=True,
                            )
                            nc.vector.tensor_copy(fo[:, ot * 512 : (ot + 1) * 512], fp_[:])
                        row0 = b * SEQ + q0 + qh * 512 + qq * 128
                        nc.sync.dma_start(outp[row0 : row0 + 128, :], fo[:])

    nc.compile()
    return nc


def _prep_inputs(inputs, W_qkv, W_fc, padding_mask):
    x2 = np.ascontiguousarray(np.asarray(inputs, np.float32).reshape(ROWS, DIM))
    xT = np.ascontiguousarray(x2.T)
    Wq = np.asarray(W_qkv, np.float32)
    Wf = np.asarray(W_fc, np.float32)
    keep_full = (np.asarray(padding_mask) == 0).astype(np.float32)  # [B, SEQ]
    keepr = np.ascontiguousarray(keep_full.reshape(B, 16, 128).transpose(0, 2, 1))
    id128 = np.eye(128, dtype=np.float32)
    eAv = np.zeros((1, 128), np.float32)
    eAv[0, :64] = 1.0
    eBv = np.zeros((1, 128), np.float32)
    eBv[0, 64:] = 1.0
    in_maps = []
    for i in range(8):
        h0 = 2 * i
        rows = np.concatenate(
            [
                Wq[h0 * 64 : (h0 + 2) * 64],
                Wq[DIM + h0 * 64 : DIM + (h0 + 2) * 64],
                Wq[2 * DIM + h0 * 64 : 2 * DIM + (h0 + 2) * 64],
            ],
            axis=0,
        )  # [384, 1024]
        in_maps.append(
            {
                "xT": xT,
                "wqkvT": np.ascontiguousarray(rows.T),
                "wfcT": np.ascontiguousarray(Wf[:, i * 128 : (i + 1) * 128].T),
                "keep": keepr,
                "id128": id128,
                "eA": eAv,
                "eB": eBv,
            }
        )
    return in_maps


def kernel(inputs, W_qkv, W_fc, b_fc, padding_mask, trace=False, trace_kwargs=None):
    global LAST_RESULTS
    if "nc" not in _CACHE:
        _CACHE["nc"] = _build()
    nc = _CACHE["nc"]
    in_maps = _prep_inputs(inputs, W_qkv, W_fc, padding_mask)
    kw = {}
    if trace:
        kw["trace"] = True
        if trace_kwargs:
            kw.update(trace_kwargs)
    res = run_bass_kernel_spmd(nc, in_maps, core_ids=list(range(8)), **kw)
    LAST_RESULTS = res
    acc = np.zeros((ROWS, DIM), np.float64)
    for r in res.results:
        acc += r["outp"].astype(np.float64)
    acc += np.asarray(b_fc, np.float64)[None, :]
    return acc.astype(np.float32).reshape(B, SEQ, DIM)



# revision 71
# speedup vs baseline: 1.3783x; 1.3783x over previous
"""Trainium2 Bass kernel for nn_Attention (B=4, SEQ=2048, DIM=1024, H=16).

Sharding: tensor-parallel over heads - 2 heads per core on 8 cores.
Per core: QKV projection (its heads), attention, row-parallel FC partial.
Gather: host sums the 8 partial FC outputs (+ b_fc).

This is the proven dense kernel plus host-side key compaction: each
batch's rows are reordered kept-first on the host (the padding mask only
masks KEYS; all rows remain queries and the row permutation is undone on
the host for free). K/V, scores, exp and AV then only cover
ceil(kept/128)*128 key slots (~half of 2048), roughly halving the
dominant exp (Act engine) and score/AV (PE) work. All device-side
instruction patterns are identical to the dense kernel.
"""

import sys

sys.path.insert(0, "/opt/trn_rl_repo")

from contextlib import ExitStack

import numpy as np

import concourse.bass as bass
import concourse.tile as tile
from concourse import bacc, mybir
from concourse.bass_utils import run_bass_kernel_spmd

F32 = mybir.dt.float32
F32R = mybir.dt.float32r
BF16 = mybir.dt.bfloat16

B, SEQ, DIM, H, DH = 4, 2048, 1024, 16, 64
ROWS = B * SEQ  # 8192
TMAX = 16
SCALE = DH ** -0.5  # 0.125

_CACHE = {}
LAST_RESULTS = None


def _build(Ts):
    Ts = tuple(Ts)
    nc = bacc.Bacc(
        "TRN2",
        target_bir_lowering=False,
        debug=False,
        enable_asserts=False,
        num_devices=8,
    )
    xT = nc.dram_tensor("xT", [DIM, ROWS], F32R, kind="ExternalInput").ap()
    wqkvT = nc.dram_tensor("wqkvT", [DIM, 384], F32R, kind="ExternalInput").ap()
    wfcT = nc.dram_tensor("wfcT", [128, DIM], F32R, kind="ExternalInput").ap()
    keep = nc.dram_tensor("keep", [B, 128, TMAX], F32, kind="ExternalInput").ap()
    id128 = nc.dram_tensor("id128", [128, 128], F32, kind="ExternalInput").ap()
    eA = nc.dram_tensor("eA", [1, 128], F32R, kind="ExternalInput").ap()
    eB = nc.dram_tensor("eB", [1, 128], F32R, kind="ExternalInput").ap()
    outp = nc.dram_tensor("outp", [ROWS, DIM], F32, kind="ExternalOutput").ap()

    EXP = mybir.ActivationFunctionType.Exp

    with tile.TileContext(nc) as tc, ExitStack() as ctx:
        p_const = ctx.enter_context(tc.tile_pool(name="const", bufs=1))
        p_xin = ctx.enter_context(tc.tile_pool(name="xin", bufs=10))
        p_qk = ctx.enter_context(tc.tile_pool(name="qk", bufs=1))
        p_vt = ctx.enter_context(tc.tile_pool(name="vt", bufs=1))
        p_va = ctx.enter_context(tc.tile_pool(name="va", bufs=2))
        p_pt = ctx.enter_context(tc.tile_pool(name="pt", bufs=40))
        p_xn = ctx.enter_context(tc.tile_pool(name="xn", bufs=3))
        p_rsb = ctx.enter_context(tc.tile_pool(name="rsb", bufs=2))
        p_r = ctx.enter_context(tc.tile_pool(name="r", bufs=1))
        p_fco = ctx.enter_context(tc.tile_pool(name="fco", bufs=3))
        p_st = ctx.enter_context(tc.tile_pool(name="st", bufs=2, space="PSUM"))
        p_xa = ctx.enter_context(tc.tile_pool(name="xa", bufs=2, space="PSUM"))
        p_mm = ctx.enter_context(tc.tile_pool(name="mm", bufs=2, space="PSUM"))

        wqkv_sb = p_const.tile([128, 8 * 384], F32R, tag="wqkv")
        for c in range(8):
            nc.sync.dma_start(
                wqkv_sb[:, c * 384 : (c + 1) * 384],
                wqkvT[c * 128 : (c + 1) * 128, :],
            )
        wfc_sb = p_const.tile([128, DIM], F32R, tag="wfc")
        nc.sync.dma_start(wfc_sb[:], wfcT[:])
        keep_sb = p_const.tile([128, B * TMAX], F32, tag="keep")
        for b in range(B):
            nc.sync.dma_start(keep_sb[:, b * TMAX : (b + 1) * TMAX], keep[b])
        id_sb = p_const.tile([128, 128], F32, tag="id")
        nc.sync.dma_start(id_sb[:], id128[:])
        ea_sb = p_const.tile([1, 128], F32R, tag="ea")
        nc.sync.dma_start(ea_sb[:], eA[:])
        eb_sb = p_const.tile([1, 128], F32R, tag="eb")
        nc.sync.dma_start(eb_sb[:], eB[:])

        for b in range(B):
            T = Ts[b]
            # ---- QKV projection: q for all rows; k/v only for kept rows
            qT2 = p_qk.tile([128, SEQ], F32R, tag="q")
            kT2 = p_qk.tile([128, SEQ], F32R, tag="k")
            vT2 = p_vt.tile([128, SEQ], F32, tag="v")
            dsts = [qT2, kT2, vT2]
            for n in range(4):
                xins = []
                for c in range(8):
                    xt = p_xin.tile([128, 512], F32R, tag="xin")
                    nc.sync.dma_start(
                        xt[:],
                        xT[
                            c * 128 : (c + 1) * 128,
                            b * SEQ + n * 512 : b * SEQ + (n + 1) * 512,
                        ],
                    )
                    xins.append(xt)
                for m in range(3):
                    # k/v are only consumed for the first T*128 kept rows
                    if m > 0 and n * 512 >= T * 128:
                        continue
                    ps = p_mm.tile([128, 512], F32, tag="mm")
                    for c in range(8):
                        nc.tensor.matmul(
                            ps[:],
                            wqkv_sb[
                                :, c * 384 + m * 128 : c * 384 + (m + 1) * 128
                            ],
                            xins[c][:],
                            start=(c == 0),
                            stop=(c == 7),
                        )
                    nc.vector.tensor_copy(dsts[m][:, n * 512 : (n + 1) * 512], ps[:])

            # ---- v-transpose + keep-scaled augmented V  [128k, T*(65+65)]
            va = p_va.tile([128, TMAX * 130], BF16, tag="va")
            for kj in range(T):
                tp = p_mm.tile([128, 128], F32, tag="mm")
                nc.tensor.transpose(tp[:], vT2[:, kj * 128 : (kj + 1) * 128], id_sb[:])
                kap = keep_sb[:, b * TMAX + kj : b * TMAX + kj + 1]
                o = kj * 130
                nc.vector.tensor_scalar_mul(va[:, o : o + 64], tp[:, 0:64], kap)
                nc.vector.tensor_copy(va[:, o + 64 : o + 65], kap)
                nc.vector.tensor_scalar_mul(va[:, o + 65 : o + 129], tp[:, 64:128], kap)
                nc.vector.tensor_copy(va[:, o + 129 : o + 130], kap)

            # ---- attention + FC per 1024-query tile
            for qt in range(2):
                q0 = qt * 1024
                pts = {}
                for a in range(2):
                    for kj in range(T):
                        st = p_st.tile([128, 1024], F32, tag="st")
                        for hh in range(2):
                            nc.tensor.matmul(
                                st[:, hh * 512 : (hh + 1) * 512],
                                kT2[
                                    a * 64 : (a + 1) * 64, kj * 128 : (kj + 1) * 128
                                ],
                                qT2[
                                    a * 64 : (a + 1) * 64,
                                    q0 + hh * 512 : q0 + (hh + 1) * 512,
                                ],
                                start=True,
                                stop=True,
                            )
                        pt = p_pt.tile([128, 1024], BF16, tag="pt")
                        nc.scalar.activation(pt[:], st[:], EXP, scale=SCALE)
                        pts[(a, kj)] = pt

                for qh in range(2):
                    xaugs = []
                    for a in range(2):
                        xa = p_xa.tile([65, 512], F32, tag="xa")
                        for kj in range(T):
                            o = kj * 130 + a * 65
                            nc.tensor.matmul(
                                xa[:],
                                va[:, o : o + 65],
                                pts[(a, kj)][:, qh * 512 : (qh + 1) * 512],
                                start=(kj == 0),
                                stop=(kj == T - 1),
                            )
                        xaugs.append(xa)
                    # stage PSUM->SBUF (DMA cannot read PSUM)
                    xasA = p_xn.tile([65, 512], F32, tag="xasA")
                    nc.vector.tensor_copy(xasA[:], xaugs[0][:])
                    xasB = p_xn.tile([65, 512], F32, tag="xasB")
                    nc.vector.tensor_copy(xasB[:], xaugs[1][:])
                    # normalization: R[p,q] = 1/rowsum of head(p)
                    rA = p_r.tile([1, 512], F32, tag="ra")
                    nc.sync.dma_start(rA[:], xasA[64:65, :])
                    rB = p_r.tile([1, 512], F32, tag="rb")
                    nc.sync.dma_start(rB[:], xasB[64:65, :])
                    rAi = p_r.tile([1, 512], F32, tag="rai")
                    nc.vector.reciprocal_approx_fast(rAi[:], rA[:])
                    rBi = p_r.tile([1, 512], F32, tag="rbi")
                    nc.vector.reciprocal_approx_fast(rBi[:], rB[:])
                    rAc = p_r.tile([1, 512], F32R, tag="rac")
                    nc.vector.tensor_copy(rAc[:], rAi[:])
                    rBc = p_r.tile([1, 512], F32R, tag="rbc")
                    nc.vector.tensor_copy(rBc[:], rBi[:])
                    Rp = p_mm.tile([128, 512], F32, tag="mm")
                    nc.tensor.matmul(
                        Rp[:], ea_sb[:], rAc[:],
                        start=True, stop=False,
                    )
                    nc.tensor.matmul(
                        Rp[:], eb_sb[:], rBc[:],
                        start=False, stop=True,
                    )
                    Rs = p_rsb.tile([128, 512], F32R, tag="rs")
                    nc.vector.tensor_copy(Rs[:], Rp[:])
                    xn = p_xn.tile([128, 512], F32R, tag="xn")
                    nc.vector.tensor_copy(xn[0:64, :], xasA[0:64, :])
                    nc.sync.dma_start(xn[64:128, :].bitcast(F32), xasB[0:64, :])
                    nc.vector.tensor_mul(xn[:], xn[:], Rs[:])

                    # FC partial: out[q,:] = xn^T @ wfcT
                    for qq in range(4):
                        fo = p_fco.tile([128, DIM], F32, tag="fo")
                        for ot in range(2):
                            fp_ = p_mm.tile([128, 512], F32, tag="mm")
                            nc.tensor.matmul(
                                fp_[:],
                                xn[:, qq * 128 : (qq + 1) * 128],
                                wfc_sb[:, ot * 512 : (ot + 1) * 512],
                                start=True,
                                stop=True,
                            )
                            nc.vector.tensor_copy(fo[:, ot * 512 : (ot + 1) * 512], fp_[:])
                        row0 = b * SEQ + q0 + qh * 512 + qq * 128
                        nc.sync.dma_start(outp[row0 : row0 + 128, :], fo[:])

    nc.compile()
    return nc


def _prep_inputs(inputs, W_qkv, W_fc, padding_mask):
    x2 = np.ascontiguousarray(np.asarray(inputs, np.float32).reshape(ROWS, DIM))
    keep_full = np.asarray(padding_mask) == 0  # True = attendable key
    Ts = []
    perms = []
    keepr = np.zeros((B, 128, TMAX), np.float32)
    for b in range(B):
        kb = keep_full[b]
        idx_keep = np.flatnonzero(kb)
        idx_mask = np.flatnonzero(~kb)
        cnt = len(idx_keep)
        T = min(TMAX, max(1, -(-cnt // 128)))
        kc = np.zeros(TMAX * 128, np.float32)
        if cnt == 0:
            kc[: T * 128] = 1.0  # degenerate all-masked batch: avoid 0-sums
        else:
            kc[:cnt] = 1.0
        keepr[b] = kc.reshape(TMAX, 128).T
        Ts.append(T)
        perms.append(np.concatenate([idx_keep, idx_mask]) + b * SEQ)
    Ts = tuple(Ts)
    perm = np.concatenate(perms)  # device row j = full row perm[j]
    xp = x2[perm]
    xT = np.ascontiguousarray(xp.T)
    Wq = np.asarray(W_qkv, np.float32)
    Wf = np.asarray(W_fc, np.float32)
    id128 = np.eye(128, dtype=np.float32)
    eAv = np.zeros((1, 128), np.float32)
    eAv[0, :64] = 1.0
    eBv = np.zeros((1, 128), np.float32)
    eBv[0, 64:] = 1.0
    in_maps = []
    for i in range(8):
        h0 = 2 * i
        rows = np.concatenate(
            [
                Wq[h0 * 64 : (h0 + 2) * 64],
                Wq[DIM + h0 * 64 : DIM + (h0 + 2) * 64],
                Wq[2 * DIM + h0 * 64 : 2 * DIM + (h0 + 2) * 64],
            ],
            axis=0,
        )  # [384, 1024]
        in_maps.append(
            {
                "xT": xT,
                "wqkvT": np.ascontiguousarray(rows.T),
                "wfcT": np.ascontiguousarray(Wf[:, i * 128 : (i + 1) * 128].T),
                "keep": keepr,
                "id128": id128,
                "eA": eAv,
                "eB": eBv,
            }
        )
    return Ts, perm, in_maps


def kernel(inputs, W_qkv, W_fc, b_fc, padding_mask, trace=False, trace_kwargs=None):
    global LAST_RESULTS
    Ts, perm, in_maps = _prep_inputs(inputs, W_qkv, W_fc, padding_mask)
    if Ts not in _CACHE:
        _CACHE[Ts] = _build(Ts)
        _CACHE["nc"] = _CACHE[Ts]  # test.py TimelineSim fallback hook
    nc = _CACHE[Ts]
    kw = {}
    if trace:
        kw["trace"] = True
        if trace_kwargs:
            kw.update(trace_kwargs)
    res = run_bass_kernel_spmd(nc, in_maps, core_ids=list(range(8)), **kw)
    LAST_RESULTS = res
    acc = np.zeros((ROWS, DIM), np.float64)
    for r in res.results:
        acc += r["outp"].astype(np.float64)
    acc += np.asarray(b_fc, np.float64)[None, :]
    out = np.empty((ROWS, DIM), np.float32)
    out[perm] = acc.astype(np.float32)
    return out.reshape(B, SEQ, DIM)
